# revision 2
# baseline (speedup 1.0000x reference)
"""Detail-loss kernel for TRN2 (8 NeuronCores).

Reference computation (algebraically reduced):
  views = reshape(inputs, (98, 3, 256, 256)); d = infer - ref
  S[n] = sum_c d[n, c]                       (per-view 256x256 plane)
  loss = ( sum |S[n,h,w+1] - S[n,h,w-1]|     (zero-padded outside)
         + sum |S[n,h+1,w] - S[n,h-1,w]| ) / (4 * 98 * 258 * 256)

Sharding: 98 views padded to 104, 13 views per core (zero views add 0).

Per-core pipeline (h-folded layout: h = 2p + s, tile [128p, (s,w)]):
  PE    : S = I@a0 + I@a1 + I@a2 - I@b0 - I@b1 - I@b2  (f32r matmuls, PSUM)
  DVE   : copy S PSUM -> SBUF f32r tile padded with zero cols at w = -1, 256
  DVE   : gw = S[:, :, +1] - S[:, :, -1]   (free-axis shifted TT, covers edges)
  PE    : ghe = E^T @ S_odd ; gho = O^T @ S_even  (bidiagonal f32r matmuls,
          cover the h-edge rows exactly)
  ACT   : Abs + accum_out -> per-partition partial sums (3 cols per view)
Host: sum partials in float64, scale.
"""
import numpy as np
import concourse.bass as bass
import concourse.mybir as mybir
from concourse import bacc
from concourse.tile import TileContext
from concourse.bass_utils import run_bass_kernel_spmd

N_CORES = 8
V = 13                 # views per core (98 -> 104 padded)
GROUPS = [4, 4, 4, 1]  # view-group sizes for DMA batching
C, H, W = 3, 256, 256
NCOL = 3 * V
SCALE = 1.0 / (4.0 * 98.0 * 258.0 * 256.0)

_cache = {}


def _weights():
    I = np.eye(128, dtype=np.float32)
    E = (np.eye(128) - np.eye(128, k=1)).astype(np.float32)   # out[p]=in[p]-in[p-1]
    O = (np.eye(128, k=-1) - np.eye(128)).astype(np.float32)  # out[p]=in[p+1]-in[p]
    return np.stack([I, -I, E, O])


def _build():
    if "nc" in _cache:
        return _cache["nc"]
    f32, f32r = mybir.dt.float32, mybir.dt.float32r
    AluOp = mybir.AluOpType
    Act = mybir.ActivationFunctionType

    nc = bacc.Bacc(None, target_bir_lowering=False)
    a = nc.declare_dram_parameter("a", [V, C, H, W], f32r, isOutput=False)
    b = nc.declare_dram_parameter("b", [V, C, H, W], f32r, isOutput=False)
    w = nc.declare_dram_parameter("w", [4, 128, 128], f32r, isOutput=False)
    y = nc.declare_dram_parameter("y", [128, NCOL], f32, isOutput=True)

    with TileContext(nc) as tc:
        with (
            tc.tile_pool(name="wp", bufs=1) as wpool,
            tc.tile_pool(name="planes", bufs=2) as ppool,
            tc.tile_pool(name="sp", bufs=4) as spool,
            tc.tile_pool(name="scr", bufs=4) as cpool,
            tc.tile_pool(name="accp", bufs=1) as apool,
            tc.tile_pool(name="psS", bufs=4, space="PSUM") as psSp,
            tc.tile_pool(name="psG", bufs=4, space="PSUM") as psGp,
        ):
            wt = wpool.tile([128, 4, 128], f32r)
            nc.sync.dma_start(out=wt[:], in_=w.rearrange("k p m -> p k m"))
            tI, tIn, tE, tO = wt[:, 0, :], wt[:, 1, :], wt[:, 2, :], wt[:, 3, :]

            acc = apool.tile([128, NCOL], f32)

            v0 = 0
            for G in GROUPS:
                ta = [ppool.tile([128, G, 512], f32r, tag=f"ta{c}", name=f"ta{c}") for c in range(C)]
                tb = [ppool.tile([128, G, 512], f32r, tag=f"tb{c}", name=f"tb{c}") for c in range(C)]
                for c in range(C):
                    nc.sync.dma_start(
                        out=ta[c][:],
                        in_=a[v0 : v0 + G, c].rearrange("g (p s) w -> p g (s w)", s=2),
                    )
                    nc.sync.dma_start(
                        out=tb[c][:],
                        in_=b[v0 : v0 + G, c].rearrange("g (p s) w -> p g (s w)", s=2),
                    )
                pss = [psSp.tile([128, 512], f32, tag="pss", name="pss") for _ in range(G)]
                # all +I matmuls back-to-back, then all -I (minimal weight churn)
                for c in range(C):
                    for g in range(G):
                        nc.tensor.matmul(
                            pss[g][:], tI, ta[c][:, g, :], start=(c == 0), stop=False
                        )
                for c in range(C):
                    for g in range(G):
                        nc.tensor.matmul(
                            pss[g][:], tIn, tb[c][:, g, :],
                            start=False, stop=(c == C - 1),
                        )
                for g in range(G):
                    v = v0 + g
                    st = spool.tile([128, 2, 258], f32r, tag="st")
                    nc.gpsimd.memset(st[:, :, 0:1].bitcast(f32), 0.0)
                    nc.gpsimd.memset(st[:, :, 257:258].bitcast(f32), 0.0)
                    nc.vector.tensor_copy(
                        st[:, :, 1:257], pss[g][:].rearrange("p (s w) -> p s w", s=2)
                    )
                    gwt = cpool.tile([128, 512], f32, tag="gw")
                    nc.vector.tensor_tensor(
                        gwt[:].rearrange("p (s w) -> p s w", s=2),
                        st[:, :, 2:258],
                        st[:, :, 0:256],
                        AluOp.subtract,
                    )
                    psg = psGp.tile([128, 512], f32, tag="psg")
                    nc.tensor.matmul(psg[:, 0:256], tE, st[:, 1, 1:257], start=True, stop=True)
                    nc.tensor.matmul(psg[:, 256:512], tO, st[:, 0, 1:257], start=True, stop=True)
                    scr = cpool.tile([128, 512], f32, tag="scr")
                    scg = cpool.tile([128, 512], f32, tag="scg")
                    nc.scalar.activation(
                        scr[:], gwt[:], Act.Abs, accum_out=acc[:, 3 * v : 3 * v + 1]
                    )
                    nc.scalar.activation(
                        scg[:, 0:256], psg[:, 0:256], Act.Abs,
                        accum_out=acc[:, 3 * v + 1 : 3 * v + 2],
                    )
                    nc.scalar.activation(
                        scg[:, 256:512], psg[:, 256:512], Act.Abs,
                        accum_out=acc[:, 3 * v + 2 : 3 * v + 3],
                    )
                v0 += G

            nc.sync.dma_start(out=y[:], in_=acc[:])

    nc.finalize()
    _cache["nc"] = nc
    return nc


def _run(infer, ref, trace=False, trace_kwargs=None):
    nc = _build()
    x = np.ascontiguousarray(infer.reshape(98, C, H, W))
    r = np.ascontiguousarray(ref.reshape(98, C, H, W))
    pad = np.zeros((6, C, H, W), np.float32)
    x = np.concatenate([x, pad], axis=0)
    r = np.concatenate([r, pad], axis=0)
    wmat = _weights()
    in_maps = [
        {"a": x[i * V : (i + 1) * V], "b": r[i * V : (i + 1) * V], "w": wmat}
        for i in range(N_CORES)
    ]
    kwargs = {}
    if trace:
        kwargs["trace"] = True
        if trace_kwargs:
            kwargs["trace_kwargs"] = trace_kwargs
    out = run_bass_kernel_spmd(nc, in_maps, core_ids=list(range(N_CORES)), **kwargs)
    total = 0.0
    for res in out.results:
        total += res["y"].astype(np.float64).sum()
    loss = np.float32(total * SCALE)
    return loss, out


def kernel(infer, ref):
    loss, _ = _run(infer, ref)
    return np.asarray(loss, dtype=np.float32)
